# revision 25
# baseline (speedup 1.0000x reference)
"""Trainium2 Bass kernel for the Engram module (hashed n-gram memory).

Contract: kernel(**inputs) takes FULL unsharded numpy inputs and returns the
FULL output (4, 2048, 2048) f32.

Sharding (hardcoded): data parallel over tokens -- 8 cores x 1024 tokens
(core c -> batch c//2, seq half c%2); the 12 embedding tables replicated per
core in fp8 (x16 scale); no collectives. Host computes the n-gram hash
indices while sharding and adds the f32 residual + conv bias during unshard.

The kernel is Pool-engine bound: 96 indirect gathers (12 slots x 8 token
tiles) at ~1.4us each of SWDGE emission time dominate.  (One indirect DMA
can only carry ONE index per partition -- the [128, k] offset form returns
garbage on HW, verified empirically.)  Everything else is organized to hide
under the gather span and to minimize startup + tail:
  * identity matrix shipped from host (make_identity costs ~1.5us of Pool
    time before the first gather can start)
  * uniform 8x128 window (1024 cols = 2 left-context + 1022 tokens); the
    last 2 tokens of each core are patched on the host in full precision
  * dot(q, mem) fused into one DVE scalar_tensor_tensor pass per PSUM chunk
    (accum_out); sigmoid merges the two partials via its bias AP; DVE reads
    the fp8 gather output directly (verified on HW)
  * causal conv folded into the value matmul (3 host-prescaled W_v*conv_w
    fp8 matrices accumulated in PSUM); phase B split into 4 column chunks
    (512/256/128/128) so the post-last-gather tail is only the 128-col
    chunk: ~12us.

Device pipeline per core:
  phase A (8 token tiles of 128 window cols):
    - 12 single-index indirect DMAs gather fp8 embedding rows -> mem8
    - q = hs @ W_q on PE, fp8 DoubleRow (psum 512 + 256)
    - dot via DVE scalar_tensor_tensor accum_out; ACT sigmoid -> alpha
    - amb = alpha*mem (ACT, fp8->bf16); PE transposes -> amt fp8 [768, cols]
  phase B (4 col chunks x 16 hid tiles): out = sum_k (W_v*w_k)^T amt
    shifted, 9 accum matmuls fp8 DR -> ACT scale -> out_sb -> batched DMA.
"""

import os
import numpy as np
import ml_dtypes

# ---------------- problem constants (hardcoded per the contract) -------------
B, S, HID = 4, 2048, 2048
TABLE, EMB = 200000, 64
ORDERS, HEADS = 3, 4
NSLOT = ORDERS * HEADS            # 12
MEMD = NSLOT * EMB                # 768
ZROW = NSLOT * TABLE              # all-zeros pad row
TABROWS = ZROW + 4
KCONV = 3
NCORES = 8
TOK = 1024                        # tokens per core (last 2 host-patched)
CTX = 2                           # left context (conv taps)
NTILE = 8                         # 8 uniform 128-col window tiles
WIN = 1024                        # window cols; col c <-> token t0-2+c
NDEV = TOK - CTX                  # 1022 device-computed tokens per core
NKP = HID // 256                  # 8 K-pair planes for Q
MKP = MEMD // 256                 # 3 K-pair planes for V
NHID = HID // 128                 # 16 hid tiles

SCALE_TAB = 16.0
SCALE_WQ = 32.0
SCALE_WV = 128.0
SIG_SCALE = 1.0 / (float(np.sqrt(np.float64(MEMD))) * SCALE_TAB * SCALE_WQ)
OUT_SCALE = 1.0 / (SCALE_TAB * SCALE_WV)

HEAD_MULTS = np.array([2654435761, 2246822519, 3266489917, 668265263],
                      dtype=np.uint32)
POLY = np.uint32(1000003)

_BF16 = ml_dtypes.bfloat16
_FP8 = ml_dtypes.float8_e4m3

# B chunk out-token ranges [T0, T1) (chunk j emitted after tile LASTTILE[j]).
# Chunks 0,1 run mid-span with the conv folded into W_v (9 accum matmuls);
# the two chunks that trail the last gathers are unfolded (3 matmuls) with
# the conv on the otherwise-idle DVE, to shrink the tail.
T0 = (0, 510, 766, 894)
T1 = (510, 766, 894, 1022)
LASTTILE = (3, 5, 6, 7)
NCHUNK = 4
FOLDC = tuple(c == "1" for c in os.environ.get("KFC", "1100"))


def _order_acc(input_ids: np.ndarray):
    """(B, S) -> list of 3 (B, S) uint32 poly-hash accumulators."""
    Bb, Ss = input_ids.shape
    u = input_ids.astype(np.uint32)
    accs = []
    for n in range(2, 2 + ORDERS):
        pad = np.zeros((Bb, Ss + n - 1), np.uint32)
        pad[:, n - 1:] = u
        acc = np.zeros((Bb, Ss), np.uint32)
        for j in range(n):
            acc = acc * POLY + pad[:, j:j + Ss]
        accs.append(acc)
    return accs


def _global_rows(input_ids: np.ndarray) -> np.ndarray:
    """(B, S) -> (B, S, 12) int32 global row ids into the stacked table."""
    accs = _order_acc(input_ids)
    per_order = []
    for o in range(ORDERS):
        idx = (accs[o][..., None] * HEAD_MULTS[None, None, :]) \
            % np.uint32(TABLE)
        per_order.append(idx.astype(np.int32))
    gidx = np.stack(per_order, axis=2).reshape(*input_ids.shape, NSLOT)
    gidx = gidx + (np.arange(NSLOT, dtype=np.int32) * TABLE)[None, None, :]
    return gidx


# ---------------- device program ---------------------------------------------
_NC_CACHE: dict = {}


def _build_nc():
    _key = "nc"
    if _key in _NC_CACHE:
        return _NC_CACHE[_key]

    from contextlib import ExitStack

    import concourse.bass as bass
    import concourse.mybir as mybir
    import concourse.tile as tile
    from concourse import bacc

    f32 = mybir.dt.float32
    bf16 = mybir.dt.bfloat16
    fp8 = mybir.dt.float8e4
    i32 = mybir.dt.int32
    MULT = mybir.AluOpType.mult
    AF = mybir.ActivationFunctionType
    DR = mybir.MatmulPerfMode.DoubleRow

    nc = bacc.Bacc("TRN2", target_bir_lowering=False, debug=False,
                   enable_asserts=False, num_devices=NCORES)

    tab = nc.dram_tensor("tab8", [TABROWS, EMB], fp8,
                         kind="ExternalInput").ap()
    hst = nc.dram_tensor("hst8", [NKP * 128, 2 * WIN], fp8,
                         kind="ExternalInput").ap()
    wq = nc.dram_tensor("wq8", [NKP * 128, 2 * MEMD], fp8,
                        kind="ExternalInput").ap()
    wvk = nc.dram_tensor("wvk8", [MKP * 128, KCONV * 2 * HID], fp8,
                         kind="ExternalInput").ap()
    wv = nc.dram_tensor("wv8", [MKP * 128, 2 * HID], fp8,
                        kind="ExternalInput").ap()
    cwt = nc.dram_tensor("cw", [128, NHID * KCONV], f32,
                         kind="ExternalInput").ap()
    idxs = nc.dram_tensor("idxs", [128, NTILE * NSLOT], i32,
                          kind="ExternalInput").ap()
    iden = nc.dram_tensor("iden", [128, 128], bf16,
                          kind="ExternalInput").ap()
    outT = nc.dram_tensor("outT", [HID, TOK], bf16, kind="ExternalOutput").ap()

    with tile.TileContext(nc) as tc, ExitStack() as ctx:
        pool = lambda name, bufs, space="SBUF": ctx.enter_context(
            tc.tile_pool(name=name, bufs=bufs, space=space))

        p_w = pool("w", 1)
        p_amt = pool("amt", 1)
        p_mem = pool("mem", 6)
        p_amb = pool("amb", 2)
        p_scr = pool("scr", 2)
        p_sc = pool("sc", 4)
        p_out = pool("out", 2)
        p_qps = pool("qps", 2, space="PSUM")
        p_tp = pool("tp", 2, space="PSUM")
        p_pv = pool("pv", 2, space="PSUM")

        # resident loads (sync queue; idx first so gathers start immediately)
        idx_sb = p_w.tile([128, NTILE * NSLOT], i32, name="idx_sb")
        nc.sync.dma_start(idx_sb[:], idxs[:, :])
        identb = p_w.tile([128, 128], bf16, name="identb")
        nc.sync.dma_start(identb[:], iden[:, :])
        wq_sb = p_w.tile([128, NKP, 2, MEMD], fp8, name="wq_sb")
        nc.sync.dma_start(
            wq_sb[:],
            wq.rearrange("(kp p) (pl n) -> p kp pl n", p=128, pl=2))
        hst_sb = p_w.tile([128, NKP, 2, WIN], fp8, name="hst_sb")
        nc.sync.dma_start(
            hst_sb[:],
            hst.rearrange("(kp p) (pl c) -> p kp pl c", p=128, pl=2))
        wvk_sb = []
        for kp in range(MKP):
            t = p_w.tile([128, KCONV, 2, HID], fp8, name=f"wvk_sb{kp}")
            nc.sync.dma_start(
                t[:],
                wvk[128 * kp:128 * (kp + 1), :].rearrange(
                    "p (k pl h) -> p k pl h", k=KCONV, pl=2))
            wvk_sb.append(t)
        wv_sb = []
        for kp in range(MKP):
            t = p_w.tile([128, 2, HID], fp8, name=f"wv_sb{kp}")
            nc.sync.dma_start(
                t[:],
                wv[128 * kp:128 * (kp + 1), :].rearrange(
                    "p (pl h) -> p pl h", pl=2))
            wv_sb.append(t)
        cw_sb = p_w.tile([128, NHID, KCONV], f32, name="cw_sb")
        nc.sync.dma_start(cw_sb[:], cwt.rearrange("p (m k) -> p m k",
                                                  k=KCONV))

        amt = p_amt.tile([128, MKP, 2, WIN], fp8, name="amt")
        v_sb = p_amt.tile([128, NHID, WIN], bf16, name="v_sb")

        def _emit_atile(i):
            c0 = 128 * i
            mem8 = p_mem.tile([128, MEMD], fp8, tag="mem", name=f"mem{i}")
            for j in range(NSLOT):
                nc.gpsimd.indirect_dma_start(
                    out=mem8[:, EMB * j:EMB * (j + 1)], out_offset=None,
                    in_=tab[:, :],
                    in_offset=bass.IndirectOffsetOnAxis(
                        ap=idx_sb[:, NSLOT * i + j:NSLOT * i + j + 1],
                        axis=0))
            qA = p_qps.tile([128, 512], f32, space="PSUM", tag="qA",
                            name=f"qA{i}")
            qB = p_qps.tile([128, 256], f32, space="PSUM", tag="qB",
                            name=f"qB{i}")
            for kp in range(NKP):
                nc.tensor.matmul(qA[:, :], lhsT=hst_sb[:, kp, :, c0:c0 + 128],
                                 rhs=wq_sb[:, kp, :, 0:512],
                                 start=(kp == 0), stop=(kp == NKP - 1),
                                 perf_mode=DR)
            for kp in range(NKP):
                nc.tensor.matmul(qB[:, :], lhsT=hst_sb[:, kp, :, c0:c0 + 128],
                                 rhs=wq_sb[:, kp, :, 512:768],
                                 start=(kp == 0), stop=(kp == NKP - 1),
                                 perf_mode=DR)
            scr = p_scr.tile([128, 512], bf16, tag="scr", name=f"scr{i}")
            d1 = p_sc.tile([128, 1], f32, tag="d1", name=f"d1_{i}")
            d2 = p_sc.tile([128, 1], f32, tag="d2", name=f"d2_{i}")
            nc.vector.scalar_tensor_tensor(
                out=scr[:, 0:512], in0=qA[:, :], scalar=SIG_SCALE,
                in1=mem8[:, 0:512], op0=MULT, op1=MULT, accum_out=d1[:])
            nc.vector.scalar_tensor_tensor(
                out=scr[:, 0:256], in0=qB[:, :], scalar=SIG_SCALE,
                in1=mem8[:, 512:768], op0=MULT, op1=MULT, accum_out=d2[:])
            alpha = p_sc.tile([128, 1], f32, tag="alpha", name=f"alpha{i}")
            nc.scalar.activation(alpha[:], d1[:], AF.Sigmoid, bias=d2[:])
            amb = p_amb.tile([128, MEMD], bf16, tag="amb", name=f"amb{i}")
            nc.scalar.activation(amb[:], mem8[:], AF.Identity, scale=alpha[:])
            return amb

        def _emit_transpose(i, amb):
            c0 = 128 * i
            tp = p_tp.tile([128, MEMD // 128, 128], bf16, space="PSUM",
                           tag="tp", name=f"tp{i}")
            for mt in range(MEMD // 128):
                nc.tensor.transpose(
                    tp[:, mt, :], amb[:, 128 * mt:128 * (mt + 1)],
                    identb[:, :])
            nc.scalar.activation(
                amt[:, :, :, c0:c0 + 128].rearrange(
                    "p kp pl c -> p (kp pl) c"), tp[:, :, :], AF.Identity)

        def _emit_bchunk(j):
            t0c, tw = T0[j], T1[j] - T0[j]
            obs = [p_out.tile([128, 4, 512], bf16, tag=f"ob{g}",
                              name=f"ob{j}_{g}") for g in range(NHID // 4)]
            for mt in range(NHID):
                h0 = 128 * mt
                pv = p_pv.tile([128, 512], f32, space="PSUM", tag="pv",
                               name=f"pv{j}_{mt}")
                if FOLDC[j]:
                    n = 0
                    for k in range(KCONV):
                        for kp in range(MKP):
                            nc.tensor.matmul(
                                pv[:, 0:tw],
                                lhsT=wvk_sb[kp][:, k, :, h0:h0 + 128],
                                rhs=amt[:, kp, :, t0c + k:t0c + k + tw],
                                start=(n == 0), stop=(n == KCONV * MKP - 1),
                                perf_mode=DR)
                            n += 1
                    nc.scalar.activation(obs[mt // 4][:, mt % 4, 0:tw],
                                         pv[:, 0:tw], AF.Identity,
                                         scale=OUT_SCALE)
                    if mt % 4 == 3:
                        g = mt // 4
                        nc.sync.dma_start(
                            outT[512 * g:512 * (g + 1),
                                 t0c:t0c + tw].rearrange(
                                "(sub p2) c -> p2 sub c", p2=128),
                            obs[g][:, :, 0:tw])
                else:
                    vw = tw + CTX
                    for kp in range(MKP):
                        nc.tensor.matmul(
                            pv[:, 0:vw],
                            lhsT=wv_sb[kp][:, :, h0:h0 + 128],
                            rhs=amt[:, kp, :, t0c:t0c + vw],
                            start=(kp == 0), stop=(kp == MKP - 1),
                            perf_mode=DR)
                    nc.scalar.activation(v_sb[:, mt, t0c:t0c + vw],
                                         pv[:, 0:vw], AF.Identity,
                                         scale=OUT_SCALE)
            if not FOLDC[j]:
                _emit_cchunk(j, obs)

        def _emit_cchunk(j, obs):
            t0c, tw = T0[j], T1[j] - T0[j]
            ADD = mybir.AluOpType.add
            for mt in range(NHID):
                t1t = p_scr.tile([128, 512], bf16, tag="ct1",
                                 name=f"c1_{j}_{mt}")
                t2t = p_scr.tile([128, 512], bf16, tag="ct2",
                                 name=f"c2_{j}_{mt}")
                nc.vector.tensor_scalar(
                    out=t1t[:, 0:tw], in0=v_sb[:, mt, t0c:t0c + tw],
                    scalar1=cw_sb[:, mt, 0:1], scalar2=None, op0=MULT)
                nc.vector.scalar_tensor_tensor(
                    out=t2t[:, 0:tw], in0=v_sb[:, mt, t0c + 1:t0c + 1 + tw],
                    scalar=cw_sb[:, mt, 1:2], in1=t1t[:, 0:tw],
                    op0=MULT, op1=ADD)
                nc.vector.scalar_tensor_tensor(
                    out=obs[mt // 4][:, mt % 4, 0:tw],
                    in0=v_sb[:, mt, t0c + 2:t0c + 2 + tw],
                    scalar=cw_sb[:, mt, 2:3], in1=t2t[:, 0:tw],
                    op0=MULT, op1=ADD)
                if mt % 4 == 3:
                    g = mt // 4
                    nc.sync.dma_start(
                        outT[512 * g:512 * (g + 1), t0c:t0c + tw].rearrange(
                            "(sub p2) c -> p2 sub c", p2=128),
                        obs[g][:, :, 0:tw])

        # ---- emission: A tiles with lag-1 transposes; B chunk j right
        # after the transpose of its last needed tile ----------------------
        ambs = {}
        nextb = 0
        for i in range(NTILE):
            ambs[i] = _emit_atile(i)
            if i > 0:
                _emit_transpose(i - 1, ambs[i - 1])
                while nextb < NCHUNK and LASTTILE[nextb] == i - 1:
                    _emit_bchunk(nextb)
                    nextb += 1
        _emit_transpose(NTILE - 1, ambs[NTILE - 1])
        while nextb < NCHUNK:
            _emit_bchunk(nextb)
            nextb += 1

    nc.compile()
    _NC_CACHE[_key] = nc
    return nc


# ---------------- host-side sharding -----------------------------------------
def _make_in_maps(inputs: dict):
    hs = np.asarray(inputs["hidden_states"], dtype=np.float32)
    ids = np.asarray(inputs["input_ids"])
    tabs = np.asarray(inputs["emb_tables"], dtype=np.float32)
    W_q = np.asarray(inputs["W_q"], dtype=np.float32)
    W_v = np.asarray(inputs["W_v"], dtype=np.float32)
    conv_w = np.asarray(inputs["conv_w"], dtype=np.float32).reshape(HID, KCONV)

    tab8 = np.zeros((TABROWS, EMB), dtype=_FP8)
    tab8[:ZROW] = (tabs.reshape(ZROW, EMB) * SCALE_TAB).astype(_FP8)
    gidx = _global_rows(ids)                              # (B, S, 12) int32

    # wq8[kp*128+p, pl*768+n] = 32*W_q[256kp+128pl+p, n]
    wq8 = np.ascontiguousarray(
        (W_q.reshape(NKP, 2, 128, MEMD).transpose(0, 2, 1, 3) * SCALE_WQ)
        .astype(_FP8).reshape(NKP * 128, 2 * MEMD))
    # wvk8[kp*128+p, (k*2+pl)*2048+h] = 128*W_v[256kp+128pl+p, h]*conv_w[h,k]
    wvkf = (W_v[None, :, :] * conv_w.T[:, None, :] * SCALE_WV)  # (3,768,2048)
    wvk8 = np.ascontiguousarray(
        wvkf.reshape(KCONV, MKP, 2, 128, HID).transpose(1, 3, 0, 2, 4)
        .astype(_FP8).reshape(MKP * 128, KCONV * 2 * HID))
    # wv8[kp*128+p, pl*2048+h] = 128*W_v[256kp+128pl+p, h]
    wv8 = np.ascontiguousarray(
        (W_v.reshape(MKP, 2, 128, HID).transpose(0, 2, 1, 3) * SCALE_WV)
        .astype(_FP8).reshape(MKP * 128, 2 * HID))
    # cw[p, mt*3+k] = conv_w[mt*128+p, k]
    cw = np.ascontiguousarray(
        conv_w.reshape(NHID, 128, KCONV).transpose(1, 0, 2)
        .reshape(128, NHID * KCONV))
    iden = np.eye(128, dtype=_BF16)

    in_maps = []
    for c in range(NCORES):
        b, h = divmod(c, 2)
        t0 = h * TOK
        lo = t0 - CTX
        v0 = max(0, lo)                                   # first valid token
        nv = t0 + TOK - CTX - v0                          # valid token count
        win_idx = np.full((WIN, NSLOT), ZROW, dtype=np.int32)
        win_idx[v0 - lo:v0 - lo + nv] = gidx[b, v0:t0 + TOK - CTX]
        hsw = np.zeros((WIN, HID), dtype=np.float32)
        hsw[v0 - lo:v0 - lo + nv] = hs[b, v0:t0 + TOK - CTX]
        # hst8[kp*128+p, pl*WIN+c] = hs[tok(c), 256kp+128pl+p]
        hst8 = np.ascontiguousarray(
            hsw.reshape(WIN, NKP, 2, 128).transpose(1, 3, 2, 0)
            .astype(_FP8).reshape(NKP * 128, 2 * WIN))
        in_maps.append({
            "tab8": tab8,
            "hst8": hst8,
            "wq8": wq8,
            "wvk8": wvk8,
            "wv8": wv8,
            "cw": cw,
            "iden": iden,
            "idxs": np.ascontiguousarray(
                win_idx.reshape(NTILE, 128, NSLOT).transpose(1, 0, 2)
                .reshape(128, NTILE * NSLOT)),
        })
    return in_maps


def _tail_tokens(inputs: dict) -> np.ndarray:
    """Full-precision host compute of the module output for the last CTX
    tokens of each core's slice. Returns (NCORES, CTX, HID) f32."""
    hs = np.asarray(inputs["hidden_states"], dtype=np.float64)
    ids = np.asarray(inputs["input_ids"])
    tabs = np.asarray(inputs["emb_tables"], dtype=np.float64)
    W_q = np.asarray(inputs["W_q"], dtype=np.float64)
    W_v = np.asarray(inputs["W_v"], dtype=np.float64)
    conv_w = np.asarray(inputs["conv_w"], dtype=np.float64).reshape(HID, KCONV)
    conv_b = np.asarray(inputs["conv_b"], dtype=np.float64)

    accs = _order_acc(ids)
    out = np.empty((NCORES, CTX, HID), np.float32)
    for c in range(NCORES):
        b, h = divmod(c, 2)
        t0 = h * TOK
        # need gated values for tokens t0+TOK-CTX-KCONV+1 .. t0+TOK-1
        lo = t0 + TOK - CTX - KCONV + 1
        toks = np.arange(lo, t0 + TOK)
        mem = np.empty((len(toks), MEMD), np.float64)
        for o in range(ORDERS):
            acc = accs[o][b, toks]
            for hh in range(HEADS):
                idx = ((acc * HEAD_MULTS[hh]) % np.uint32(TABLE)).astype(
                    np.int64)
                mem[:, (o * HEADS + hh) * EMB:(o * HEADS + hh + 1) * EMB] = \
                    tabs[o, hh, idx]
        q = hs[b, toks] @ W_q
        alp = 1.0 / (1.0 + np.exp(-np.sum(q * mem, -1)
                                  / np.sqrt(np.float64(MEMD))))
        gated = alp[:, None] * (mem @ W_v)               # (CTX+K-1, HID)
        for t in range(CTX):
            tt = TOK - CTX + t                           # local token
            g0 = KCONV - 1 + t                           # gated row of token
            fused = sum(conv_w[:, k] * gated[g0 - (KCONV - 1) + k]
                        for k in range(KCONV)) + conv_b
            out[c, t] = (hs[b, t0 + tt] + fused).astype(np.float32)
    return out


def _postprocess(res, inputs: dict) -> np.ndarray:
    hs = np.asarray(inputs["hidden_states"], dtype=np.float32)
    cb = np.asarray(inputs["conv_b"], dtype=np.float32)
    tails = _tail_tokens(inputs)
    out = np.empty((B, S, HID), dtype=np.float32)
    for c in range(NCORES):
        b, h = divmod(c, 2)
        t0 = h * TOK
        outT = res.results[c]["outT"]
        out[b, t0:t0 + NDEV, :] = (hs[b, t0:t0 + NDEV, :]
                                   + outT[:, :NDEV].astype(np.float32).T + cb)
        out[b, t0 + NDEV:t0 + TOK, :] = tails[c]
    return out


def _run(inputs: dict, trace: bool = False, **kw):
    from concourse import bass_utils

    nc = _build_nc()
    in_maps = _make_in_maps(inputs)
    res = bass_utils.run_bass_kernel_spmd(
        nc, in_maps, core_ids=list(range(NCORES)), trace=trace, **kw)
    return _postprocess(res, inputs), res


def kernel(**inputs) -> np.ndarray:
    out, _ = _run(inputs, trace=False)
    return out
